# revision 10
# baseline (speedup 1.0000x reference)
"""Multi-head self-attention (B=4, T=2048, D=1024, H=16) on 8 TRN2 NeuronCores.

Sharding: core c = 2*b + j computes batch b, heads j*8..j*8+7 (tensor-parallel
over heads), and a partial projection over its 512 attention-output columns.
The host sums the two partial projections per batch. No collectives.

Per-core dataflow (all matmul inputs bf16, fp32 PSUM accumulation):
  - Heads processed in even/odd pairs. K^T and Q^T for a pair live in one
    [128, T] tile (even head's 64 features on partitions 0:64, odd on 64:128).
  - Transposed scores per pair: the even and odd head matmuls contract over
    64 partitions each and run CONCURRENTLY on PE row groups 0-1 / 2-3 via
    tile_position (0,0)/(64,0), writing adjacent PSUM banks [128, 512|512].
  - One exp ACT per kt covers both heads ([128,1024], scale=1/8 folded in).
  - attn@V per head: V_aug [t,65] (ones column -> softmax denominator in
    row 64), accumulated over kt.
  - Normalize via DVE reciprocal_approx_fast + GpSimd partition broadcast +
    DVE multiply into A^T[d, t] (bf16), then partial projection (fp32).
  - The scalar engine (exp) is the bottleneck: QKV-projection and output-
    projection matmul chunks are woven between score slots so exp work is
    available from ~15us onward and the PE never bubbles ACT.
"""

import os
from collections import deque

import numpy as np
import ml_dtypes

import concourse.mybir as mybir
from concourse import bacc
from concourse.tile import TileContext
from concourse.bass_utils import run_bass_kernel_spmd

B, T, D, H = 4, 2048, 1024, 16
HD = D // H
SCALE = HD**-0.5
P = 128
BF = mybir.dt.bfloat16
F32 = mybir.dt.float32
NBF = ml_dtypes.bfloat16

LAST_RESULT = None
_built = None


def _build():
    nc = bacc.Bacc("TRN2", target_bir_lowering=False, debug=False, num_devices=8)

    xT = nc.dram_tensor("xT", [D, T], BF, kind="ExternalInput")  # x[b].T
    wqkT = nc.dram_tensor("wqkT", [D, 1024], BF, kind="ExternalInput")  # (q|k).T shard
    wvT = nc.dram_tensor("wvT", [D, 512], BF, kind="ExternalInput")
    wpT = nc.dram_tensor("wpT", [512, D], BF, kind="ExternalInput")  # proj_w.T rows
    qkb = nc.dram_tensor("qkb", [1024], F32, kind="ExternalInput")
    vb = nc.dram_tensor("vb", [512], F32, kind="ExternalInput")
    pb = nc.dram_tensor("pb", [D], F32, kind="ExternalInput")
    yT = nc.dram_tensor("yT", [D, T], F32, kind="ExternalOutput")

    Exp = mybir.ActivationFunctionType.Exp
    mult = mybir.AluOpType.mult
    add = mybir.AluOpType.add

    with TileContext(nc) as tc:
        with (
            tc.tile_pool(name="pers", bufs=1) as pers,
            tc.tile_pool(name="small", bufs=1) as small,
        ):
            # ---- persistent tensors ----
            # K^T / Q^T feature tiles, one per head pair (even head rows 0:64,
            # odd head rows 64:128)
            kts = [pers.tile([P, T], BF, tag=f"kt{i}", name=f"kt{i}") for i in range(4)]
            qts = [pers.tile([P, T], BF, tag=f"qt{i}", name=f"qt{i}") for i in range(4)]
            # attention out, d-major, one tile per head pair
            ats = [pers.tile([P, T], BF, tag=f"at{i}", name=f"at{i}") for i in range(4)]
            V4 = pers.tile([P, 16, 8, HD + 1], BF, tag="v4")  # [t-part, tt, head, 65]

            # long-lived attention pools first: pools release in LIFO order,
            # and the phase-1 pools below must close mid-emission
            spool_cm = tc.tile_pool(name="spool", bufs=2, space="PSUM")
            spool = spool_cm.__enter__()
            epool_cm = tc.tile_pool(name="epool", bufs=2)
            epool = epool_cm.__enter__()
            pop_cm = tc.tile_pool(name="pop", bufs=1, space="PSUM")
            pop = pop_cm.__enter__()
            npool_cm = tc.tile_pool(name="npool", bufs=2)
            npool = npool_cm.__enter__()

            # ---- input loads ----
            ph1x_cm = tc.tile_pool(name="ph1x", bufs=1)
            ph1x = ph1x_cm.__enter__()
            ph1w_cm = tc.tile_pool(name="ph1w", bufs=1)
            ph1w = ph1w_cm.__enter__()
            xts = []
            wqks = []
            for dt in range(8):
                t_ = ph1x.tile([P, T], BF, tag=f"xt{dt}")
                nc.sync.dma_start(t_[:], xT.ap()[dt * P : (dt + 1) * P, :])
                xts.append(t_)
                w_ = ph1w.tile([P, 1024], BF, tag=f"wqk{dt}")
                nc.sync.dma_start(w_[:], wqkT.ap()[dt * P : (dt + 1) * P, :])
                wqks.append(w_)
            wvs = []
            for dt in range(8):
                w_ = ph1x.tile([P, 512], BF, tag=f"wv{dt}")
                nc.sync.dma_start(w_[:], wvT.ap()[dt * P : (dt + 1) * P, :])
                wvs.append(w_)
            qkb_sb = small.tile([P, 8], F32, tag="qkb")
            nc.sync.dma_start(qkb_sb[:], qkb.rearrange("(o p) -> p o", p=P))
            vb_sb = small.tile([P, 512], F32, tag="vb")
            nc.sync.dma_start(vb_sb[:], vb.ap()[None, :].to_broadcast((P, 512)))
            pb_sb = small.tile([P, 8], F32, tag="pb")
            nc.sync.dma_start(pb_sb[:], pb.rearrange("(o p) -> p o", p=P))
            # ones columns of V_aug
            nc.vector.memset(V4[:, :, :, HD : HD + 1], 1.0)

            ph1p_cm = tc.tile_pool(name="ph1p", bufs=2, space="PSUM")
            ph1p = ph1p_cm.__enter__()

            def qk_chunk(ft, c4):
                # one [128, 512] column chunk of the q/k projection, feature
                # tile ft (0-3 -> Q pair ft, 4-7 -> K pair ft-4)
                p5 = ph1p.tile([P, 512], F32, tag="p5")
                col = c4 * 512
                for dt in range(8):
                    nc.tensor.matmul(
                        p5[:],
                        lhsT=wqks[dt][:, ft * P : (ft + 1) * P],
                        rhs=xts[dt][:, col : col + 512],
                        start=(dt == 0),
                        stop=(dt == 7),
                    )
                dst = qts[ft] if ft < 4 else kts[ft - 4]
                nc.vector.tensor_scalar_add(
                    dst[:, col : col + 512], p5[:], qkb_sb[:, ft : ft + 1]
                )

            def v_chunk(tt):
                p5 = ph1p.tile([P, 512], F32, tag="p5")
                for dt in range(8):
                    nc.tensor.matmul(
                        p5[:],
                        lhsT=xts[dt][:, tt * P : (tt + 1) * P],
                        rhs=wvs[dt][:],
                        start=(dt == 0),
                        stop=(dt == 7),
                    )
                nc.vector.tensor_tensor(
                    V4[:, tt, :, 0:HD],
                    p5.rearrange("p (h e) -> p h e", e=HD),
                    vb_sb.rearrange("p (h e) -> p h e", e=HD),
                    add,
                )

            # K0 + Q0 upfront so pair 0's scores can start immediately;
            # everything else is woven between score slots below.
            for c4 in range(4):
                qk_chunk(4, c4)
            for c4 in range(4):
                qk_chunk(0, c4)
            filler = deque()
            for ft in (5, 1):
                for c4 in range(4):
                    filler.append(lambda ft=ft, c4=c4: qk_chunk(ft, c4))
            for tt in range(16):
                filler.append(lambda tt=tt: v_chunk(tt))
            for ft in (6, 2, 7, 3):
                for c4 in range(4):
                    filler.append(lambda ft=ft, c4=c4: qk_chunk(ft, c4))
            n_fill = len(filler)  # 40

            # ---- attention ----
            wps = [None] * 4
            outp = [None]
            late_cms = []
            proj_q = deque()

            def attnv_slot(st, kt):
                e_, poE_, poO_, hp_, _ = st
                nc.tensor.matmul(
                    poE_[0 : HD + 1, :],
                    lhsT=V4[:, kt, 2 * hp_, :],
                    rhs=e_[:, kt, 0:512],
                    start=(kt == 0),
                    stop=(kt == 15),
                )
                nc.tensor.matmul(
                    poO_[0 : HD + 1, :],
                    lhsT=V4[:, kt, 2 * hp_ + 1, :],
                    rhs=e_[:, kt, 512:1024],
                    start=(kt == 0),
                    stop=(kt == 15),
                )

            def emit_norm(st):
                e_, poE_, poO_, hp_, qc_ = st
                cols = slice(qc_ * 512, (qc_ + 1) * 512)
                for po_, rows in ((poE_, slice(0, 64)), (poO_, slice(64, 128))):
                    rr = npool.tile([1, 512], F32, tag="rr")
                    nc.vector.reciprocal(rr[:], po_[HD : HD + 1, :])
                    rb = npool.tile([64, 512], F32, tag="rb")
                    nc.gpsimd.partition_broadcast(rb[:], rr[:])
                    nc.vector.tensor_tensor(
                        ats[hp_][rows, cols], po_[0:HD, :], rb[:], mult
                    )

            def proj_chunk(qc, ep):
                # et pair (2*ep, 2*ep+1) of the output projection at q-block qc
                pp = spool.tile([P, 1024], F32, tag="ps")
                cols = slice(qc * 512, (qc + 1) * 512)
                for half in range(2):
                    et = 2 * ep + half
                    for dt in range(4):
                        nc.tensor.matmul(
                            pp[:, half * 512 : (half + 1) * 512],
                            lhsT=wps[dt][:, et * P : (et + 1) * P],
                            rhs=ats[dt][:, cols],
                            start=(dt == 0),
                            stop=(dt == 3),
                        )
                ob = outp[0].tile([P, 1024], F32, tag="ob")
                for half in range(2):
                    et = 2 * ep + half
                    hcols = slice(half * 512, (half + 1) * 512)
                    nc.vector.tensor_scalar_add(
                        ob[:, hcols], pp[:, hcols], pb_sb[:, et : et + 1]
                    )
                    nc.sync.dma_start(
                        yT.ap()[et * P : (et + 1) * P, cols], ob[:, hcols]
                    )

            prev = None
            iters = [(hp, qc) for qc in range(4) for hp in range(4)]
            for it, (hp, qc) in enumerate(iters):
                e = epool.tile([P, 16, 1024], BF, tag="e")
                poE = pop.tile([P, 512], F32, tag="poe")
                poO = pop.tile([P, 512], F32, tag="poo")
                qcols = slice(qc * 512, (qc + 1) * 512)
                for kt in range(16):
                    ps = spool.tile([P, 1024], F32, tag="ps")
                    nc.tensor.matmul(
                        ps[:, 0:512],
                        lhsT=kts[hp][0:64, kt * P : (kt + 1) * P],
                        rhs=qts[hp][0:64, qcols],
                        start=True,
                        stop=True,
                        tile_position=(0, 0),
                    )
                    nc.tensor.matmul(
                        ps[:, 512:1024],
                        lhsT=kts[hp][64:128, kt * P : (kt + 1) * P],
                        rhs=qts[hp][64:128, qcols],
                        start=True,
                        stop=True,
                        tile_position=(64, 0),
                    )
                    if filler:
                        filler.popleft()()
                    if prev is not None and kt >= 4:
                        attnv_slot(prev, kt - 4)
                    if proj_q and hp != 0 and kt in (7, 11):
                        proj_q.popleft()()
                    nc.scalar.activation(e[:, kt, :], ps[:], Exp, scale=SCALE)
                if prev is not None:
                    for kt in range(12, 16):
                        attnv_slot(prev, kt)
                    emit_norm(prev)
                prev = (e, poE, poO, hp, qc)

                if it == 2:
                    # all phase-1 chunks are emitted; free their pools and
                    # bring in the projection weights
                    assert not filler
                    ph1p_cm.__exit__(None, None, None)
                    ph1w_cm.__exit__(None, None, None)
                    ph1x_cm.__exit__(None, None, None)
                    wpool_cm = tc.tile_pool(name="wpool", bufs=1)
                    wpool = wpool_cm.__enter__()
                    late_cms.append(wpool_cm)
                    for dt in range(4):
                        w_ = wpool.tile([P, D], BF, tag=f"wp{dt}")
                        nc.sync.dma_start(w_[:], wpT.ap()[dt * P : (dt + 1) * P, :])
                        wps[dt] = w_
                    outp_cm = tc.tile_pool(name="outp", bufs=2)
                    outp[0] = outp_cm.__enter__()
                    late_cms.append(outp_cm)

                if hp == 3:
                    # queue the projection for this q-block (runs woven into
                    # the next q-block's iterations, after emit_norm(qc, p3))
                    for ep in range(4):
                        proj_q.append(lambda qc=qc, ep=ep: proj_chunk(qc, ep))

            # drain: last iteration's attn@V + norm + last projection block
            for kt in range(16):
                attnv_slot(prev, kt)
            emit_norm(prev)
            while proj_q:
                proj_q.popleft()()

            for cm in [*reversed(late_cms), npool_cm, pop_cm, epool_cm, spool_cm]:
                cm.__exit__(None, None, None)

    nc.compile()
    return nc


def kernel(x, qkv_w, qkv_b, proj_w, proj_b):
    global _built, LAST_RESULT
    x = np.asarray(x, np.float32)
    qkv_w = np.asarray(qkv_w, np.float32)
    qkv_b = np.asarray(qkv_b, np.float32)
    proj_w = np.asarray(proj_w, np.float32)
    proj_b = np.asarray(proj_b, np.float32)

    if _built is None:
        _built = _build()
    nc = _built

    in_maps = []
    for c in range(8):
        b, j = divmod(c, 2)
        s = j * 512
        wqkT = np.concatenate([qkv_w[s : s + 512], qkv_w[1024 + s : 1024 + s + 512]]).T
        in_maps.append(
            {
                "xT": np.ascontiguousarray(x[b].T).astype(NBF),
                "wqkT": np.ascontiguousarray(wqkT).astype(NBF),
                "wvT": np.ascontiguousarray(qkv_w[2048 + s : 2048 + s + 512].T).astype(NBF),
                "wpT": np.ascontiguousarray(proj_w[:, s : s + 512].T).astype(NBF),
                "qkb": np.concatenate([qkv_b[s : s + 512], qkv_b[1024 + s : 1024 + s + 512]]),
                "vb": np.ascontiguousarray(qkv_b[2048 + s : 2048 + s + 512]),
                "pb": proj_b if j == 0 else np.zeros_like(proj_b),
            }
        )

    trace = os.environ.get("BASS_TRACE") == "1"
    if trace:
        try:
            import antenv.axon_hooks  # noqa: F401  (needed by the axon trace path)
        except ImportError:
            trace = False
            os.environ["BASS_NEVER_TRACE"] = "1"
    res = run_bass_kernel_spmd(nc, in_maps, core_ids=list(range(8)), trace=trace)
    LAST_RESULT = res

    out = np.empty((B, T, D), np.float32)
    for b in range(B):
        out[b] = (res.results[2 * b]["yT"] + res.results[2 * b + 1]["yT"]).T
    return out


# revision 14
# speedup vs baseline: 1.2858x; 1.2858x over previous
"""Multi-head self-attention (B=4, T=2048, D=1024, H=16) on 8 TRN2 NeuronCores.

Sharding: core c = 2*b + j computes batch b, heads j*8..j*8+7 (tensor-parallel
over heads), and a partial projection over its 512 attention-output columns.
The host sums the two partial projections per batch. No collectives.

Per-core dataflow (all matmul inputs bf16, fp32 PSUM accumulation):
  - Heads processed in even/odd pairs. K^T and Q^T for a pair live in one
    [128, T] tile (even head's 64 features on partitions 0:64, odd on 64:128).
  - Transposed scores per pair: the even and odd head matmuls contract over
    64 partitions each and run CONCURRENTLY on PE row groups 0-1 / 2-3 via
    tile_position (0,0)/(64,0), writing adjacent PSUM banks [128, 512|512].
  - One exp ACT per kt covers both heads ([128,1024], scale=1/8 folded in).
  - attn@V per head: V_aug [t,65] (ones column -> softmax denominator in
    row 64), accumulated over kt.
  - Normalize via DVE reciprocal_approx_fast + GpSimd partition broadcast +
    DVE multiply into A^T[d, t] (bf16), then partial projection (fp32).
  - The scalar engine (exp) is the bottleneck: QKV-projection and output-
    projection matmul chunks are woven between score slots so exp work is
    available from ~15us onward and the PE never bubbles ACT.
"""

import os
from collections import deque

import numpy as np
import ml_dtypes

import concourse.mybir as mybir
from concourse import bacc
from concourse.tile import TileContext
from concourse.bass_utils import run_bass_kernel_spmd

B, T, D, H = 4, 2048, 1024, 16
HD = D // H
SCALE = HD**-0.5
P = 128
BF = mybir.dt.bfloat16
F32 = mybir.dt.float32
NBF = ml_dtypes.bfloat16

LAST_RESULT = None
_built = None


def _build():
    nc = bacc.Bacc("TRN2", target_bir_lowering=False, debug=False, num_devices=8)

    xT = nc.dram_tensor("xT", [D, T], BF, kind="ExternalInput")  # x[b].T
    wqkT = nc.dram_tensor("wqkT", [D, 1024], BF, kind="ExternalInput")  # (q|k).T shard
    wvT = nc.dram_tensor("wvT", [D, 512], BF, kind="ExternalInput")
    wpT = nc.dram_tensor("wpT", [512, D], BF, kind="ExternalInput")  # proj_w.T rows
    qkb = nc.dram_tensor("qkb", [1024], F32, kind="ExternalInput")
    vb = nc.dram_tensor("vb", [512], F32, kind="ExternalInput")
    pb = nc.dram_tensor("pb", [D], F32, kind="ExternalInput")
    yT = nc.dram_tensor("yT", [D, T], F32, kind="ExternalOutput")

    Exp = mybir.ActivationFunctionType.Exp
    mult = mybir.AluOpType.mult
    add = mybir.AluOpType.add

    with TileContext(nc) as tc:
        with (
            tc.tile_pool(name="pers", bufs=1) as pers,
            tc.tile_pool(name="small", bufs=1) as small,
        ):
            # ---- persistent tensors ----
            # K^T / Q^T feature tiles, one per head pair (even head rows 0:64,
            # odd head rows 64:128)
            kts = [pers.tile([P, T], BF, tag=f"kt{i}", name=f"kt{i}") for i in range(4)]
            qts = [pers.tile([P, T], BF, tag=f"qt{i}", name=f"qt{i}") for i in range(4)]
            # attention out, d-major, one tile per head pair
            ats = [pers.tile([P, T], BF, tag=f"at{i}", name=f"at{i}") for i in range(4)]
            V4 = pers.tile([P, 16, 8, HD + 1], BF, tag="v4")  # [t-part, tt, head, 65]

            # long-lived attention pools first: pools release in LIFO order,
            # and the phase-1 pools below must close mid-emission
            spool_cm = tc.tile_pool(name="spool", bufs=2, space="PSUM")
            spool = spool_cm.__enter__()
            epool_cm = tc.tile_pool(name="epool", bufs=2)
            epool = epool_cm.__enter__()
            pop_cm = tc.tile_pool(name="pop", bufs=1, space="PSUM")
            pop = pop_cm.__enter__()
            npool_cm = tc.tile_pool(name="npool", bufs=2)
            npool = npool_cm.__enter__()

            # ---- input loads ----
            ph1x_cm = tc.tile_pool(name="ph1x", bufs=1)
            ph1x = ph1x_cm.__enter__()
            ph1w_cm = tc.tile_pool(name="ph1w", bufs=1)
            ph1w = ph1w_cm.__enter__()
            xts = []
            wqks = []
            for dt in range(8):
                t_ = ph1x.tile([P, T], BF, tag=f"xt{dt}")
                nc.sync.dma_start(t_[:], xT.ap()[dt * P : (dt + 1) * P, :])
                xts.append(t_)
                w_ = ph1w.tile([P, 1024], BF, tag=f"wqk{dt}")
                nc.sync.dma_start(w_[:], wqkT.ap()[dt * P : (dt + 1) * P, :])
                wqks.append(w_)
            wvs = []
            for dt in range(8):
                w_ = ph1x.tile([P, 512], BF, tag=f"wv{dt}")
                nc.sync.dma_start(w_[:], wvT.ap()[dt * P : (dt + 1) * P, :])
                wvs.append(w_)
            qkb_sb = small.tile([P, 8], F32, tag="qkb")
            nc.sync.dma_start(qkb_sb[:], qkb.rearrange("(o p) -> p o", p=P))
            vb_sb = small.tile([P, 512], F32, tag="vb")
            nc.sync.dma_start(vb_sb[:], vb.ap()[None, :].to_broadcast((P, 512)))
            pb_sb = small.tile([P, 8], F32, tag="pb")
            nc.sync.dma_start(pb_sb[:], pb.rearrange("(o p) -> p o", p=P))
            # ones columns of V_aug
            nc.vector.memset(V4[:, :, :, HD : HD + 1], 1.0)

            ph1p_cm = tc.tile_pool(name="ph1p", bufs=2, space="PSUM")
            ph1p = ph1p_cm.__enter__()

            def qk_chunk(ft, c4):
                # one [128, 512] column chunk of the q/k projection, feature
                # tile ft (0-3 -> Q pair ft, 4-7 -> K pair ft-4)
                p5 = ph1p.tile([P, 512], F32, tag="p5")
                col = c4 * 512
                for dt in range(8):
                    nc.tensor.matmul(
                        p5[:],
                        lhsT=wqks[dt][:, ft * P : (ft + 1) * P],
                        rhs=xts[dt][:, col : col + 512],
                        start=(dt == 0),
                        stop=(dt == 7),
                    )
                dst = qts[ft] if ft < 4 else kts[ft - 4]
                nc.vector.tensor_scalar_add(
                    dst[:, col : col + 512], p5[:], qkb_sb[:, ft : ft + 1]
                )

            def v_chunk(tt):
                p5 = ph1p.tile([P, 512], F32, tag="p5")
                for dt in range(8):
                    nc.tensor.matmul(
                        p5[:],
                        lhsT=xts[dt][:, tt * P : (tt + 1) * P],
                        rhs=wvs[dt][:],
                        start=(dt == 0),
                        stop=(dt == 7),
                    )
                nc.vector.tensor_tensor(
                    V4[:, tt, :, 0:HD],
                    p5.rearrange("p (h e) -> p h e", e=HD),
                    vb_sb.rearrange("p (h e) -> p h e", e=HD),
                    add,
                )

            # K0 (full T) + Q0's first q-block upfront so pair 0's scores can
            # start immediately; everything else is woven between score slots.
            for c4 in range(4):
                qk_chunk(4, c4)
            qk_chunk(0, 0)
            filler = deque()
            for c4 in range(1, 4):
                filler.append(lambda c4=c4: qk_chunk(0, c4))
            for ft in (5, 1):
                for c4 in range(4):
                    filler.append(lambda ft=ft, c4=c4: qk_chunk(ft, c4))
            for tt in range(16):
                filler.append(lambda tt=tt: v_chunk(tt))
            for ft in (6, 2, 7, 3):
                for c4 in range(4):
                    filler.append(lambda ft=ft, c4=c4: qk_chunk(ft, c4))
            n_fill = len(filler)  # 40

            # ---- attention ----
            wps = [None] * 4
            outp = [None]
            late_cms = []
            proj_q = deque()

            def attnv_slot(st, kt):
                e_, poE_, poO_, hp_, _ = st
                nc.tensor.matmul(
                    poE_[0 : HD + 1, :],
                    lhsT=V4[:, kt, 2 * hp_, :],
                    rhs=e_[:, kt, 0:512],
                    start=(kt == 0),
                    stop=(kt == 15),
                )
                nc.tensor.matmul(
                    poO_[0 : HD + 1, :],
                    lhsT=V4[:, kt, 2 * hp_ + 1, :],
                    rhs=e_[:, kt, 512:1024],
                    start=(kt == 0),
                    stop=(kt == 15),
                )

            def emit_norm(st):
                e_, poE_, poO_, hp_, qc_ = st
                cols = slice(qc_ * 512, (qc_ + 1) * 512)
                for po_, rows in ((poE_, slice(0, 64)), (poO_, slice(64, 128))):
                    # custom-DVE reciprocal_approx_fast can't read PSUM;
                    # stage the denominator row through SBUF first
                    dsb = npool.tile([1, 512], F32, tag="dsb")
                    nc.vector.tensor_copy(dsb[:], po_[HD : HD + 1, :])
                    rr = npool.tile([1, 512], F32, tag="rr")
                    nc.vector.reciprocal_approx_fast(rr[:], dsb[:])
                    rb = npool.tile([64, 512], F32, tag="rb")
                    nc.gpsimd.partition_broadcast(rb[:], rr[:])
                    nc.vector.tensor_tensor(
                        ats[hp_][rows, cols], po_[0:HD, :], rb[:], mult
                    )

            def proj_chunk(qc, ep):
                # et pair (2*ep, 2*ep+1) of the output projection at q-block qc
                pp = spool.tile([P, 1024], F32, tag="ps")
                cols = slice(qc * 512, (qc + 1) * 512)
                for half in range(2):
                    et = 2 * ep + half
                    for dt in range(4):
                        nc.tensor.matmul(
                            pp[:, half * 512 : (half + 1) * 512],
                            lhsT=wps[dt][:, et * P : (et + 1) * P],
                            rhs=ats[dt][:, cols],
                            start=(dt == 0),
                            stop=(dt == 3),
                        )
                ob = outp[0].tile([P, 1024], F32, tag="ob")
                for half in range(2):
                    et = 2 * ep + half
                    hcols = slice(half * 512, (half + 1) * 512)
                    nc.vector.tensor_scalar_add(
                        ob[:, hcols], pp[:, hcols], pb_sb[:, et : et + 1]
                    )
                    nc.sync.dma_start(
                        yT.ap()[et * P : (et + 1) * P, cols], ob[:, hcols]
                    )

            prev = None
            pop2 = [None]
            iters = [(hp, qc) for qc in range(4) for hp in range(4)]
            for it, (hp, qc) in enumerate(iters):
                e = epool.tile([P, 16, 1024], BF, tag="e")
                # alternate between two single-buffered psum pools so the
                # norm chain of iteration i never blocks attn@V of i+1
                pool_i = pop2[0] if (pop2[0] is not None and it % 2 == 1) else pop
                poE = pool_i.tile([P, 512], F32, tag="poe")
                poO = pool_i.tile([P, 512], F32, tag="poo")
                qcols = slice(qc * 512, (qc + 1) * 512)
                for kt in range(16):
                    ps = spool.tile([P, 1024], F32, tag="ps")
                    nc.tensor.matmul(
                        ps[:, 0:512],
                        lhsT=kts[hp][0:64, kt * P : (kt + 1) * P],
                        rhs=qts[hp][0:64, qcols],
                        start=True,
                        stop=True,
                        tile_position=(0, 0),
                    )
                    nc.tensor.matmul(
                        ps[:, 512:1024],
                        lhsT=kts[hp][64:128, kt * P : (kt + 1) * P],
                        rhs=qts[hp][64:128, qcols],
                        start=True,
                        stop=True,
                        tile_position=(64, 0),
                    )
                    if filler:
                        filler.popleft()()
                    if prev is not None and kt >= 4:
                        attnv_slot(prev, kt - 4)
                    if proj_q and hp != 0 and kt in (7, 11):
                        proj_q.popleft()()
                    nc.scalar.activation(e[:, kt, :], ps[:], Exp, scale=SCALE)
                if prev is not None:
                    for kt in range(12, 16):
                        attnv_slot(prev, kt)
                    emit_norm(prev)
                prev = (e, poE, poO, hp, qc)

                if it == 2:
                    # all phase-1 chunks are emitted; free their pools and
                    # bring in the projection weights
                    assert not filler
                    ph1p_cm.__exit__(None, None, None)
                    ph1w_cm.__exit__(None, None, None)
                    ph1x_cm.__exit__(None, None, None)
                    wpool_cm = tc.tile_pool(name="wpool", bufs=1)
                    wpool = wpool_cm.__enter__()
                    late_cms.append(wpool_cm)
                    for dt in range(4):
                        w_ = wpool.tile([P, D], BF, tag=f"wp{dt}")
                        nc.sync.dma_start(w_[:], wpT.ap()[dt * P : (dt + 1) * P, :])
                        wps[dt] = w_
                    outp_cm = tc.tile_pool(name="outp", bufs=2)
                    outp[0] = outp_cm.__enter__()
                    late_cms.append(outp_cm)
                    pop2_cm = tc.tile_pool(name="pop2", bufs=1, space="PSUM")
                    pop2[0] = pop2_cm.__enter__()
                    late_cms.append(pop2_cm)

                if hp == 3:
                    # queue the projection for this q-block (runs woven into
                    # the next q-block's iterations, after emit_norm(qc, p3))
                    for ep in range(4):
                        proj_q.append(lambda qc=qc, ep=ep: proj_chunk(qc, ep))

            # drain: last iteration's attn@V + norm + last projection block
            for kt in range(16):
                attnv_slot(prev, kt)
            emit_norm(prev)
            while proj_q:
                proj_q.popleft()()

            for cm in [*reversed(late_cms), npool_cm, pop_cm, epool_cm, spool_cm]:
                cm.__exit__(None, None, None)

    nc.compile()
    return nc


def kernel(x, qkv_w, qkv_b, proj_w, proj_b):
    global _built, LAST_RESULT
    x = np.asarray(x, np.float32)
    qkv_w = np.asarray(qkv_w, np.float32)
    qkv_b = np.asarray(qkv_b, np.float32)
    proj_w = np.asarray(proj_w, np.float32)
    proj_b = np.asarray(proj_b, np.float32)

    if _built is None:
        _built = _build()
    nc = _built

    in_maps = []
    for c in range(8):
        b, j = divmod(c, 2)
        s = j * 512
        wqkT = np.concatenate([qkv_w[s : s + 512], qkv_w[1024 + s : 1024 + s + 512]]).T
        in_maps.append(
            {
                "xT": np.ascontiguousarray(x[b].T).astype(NBF),
                "wqkT": np.ascontiguousarray(wqkT).astype(NBF),
                "wvT": np.ascontiguousarray(qkv_w[2048 + s : 2048 + s + 512].T).astype(NBF),
                "wpT": np.ascontiguousarray(proj_w[:, s : s + 512].T).astype(NBF),
                "qkb": np.concatenate([qkv_b[s : s + 512], qkv_b[1024 + s : 1024 + s + 512]]),
                "vb": np.ascontiguousarray(qkv_b[2048 + s : 2048 + s + 512]),
                "pb": proj_b if j == 0 else np.zeros_like(proj_b),
            }
        )

    trace = os.environ.get("BASS_TRACE") == "1"
    if trace:
        try:
            import antenv.axon_hooks  # noqa: F401  (needed by the axon trace path)
        except ImportError:
            trace = False
            os.environ["BASS_NEVER_TRACE"] = "1"
    res = run_bass_kernel_spmd(nc, in_maps, core_ids=list(range(8)), trace=trace)
    LAST_RESULT = res

    out = np.empty((B, T, D), np.float32)
    for b in range(B):
        out[b] = (res.results[2 * b]["yT"] + res.results[2 * b + 1]["yT"]).T
    return out


# revision 18
# speedup vs baseline: 1.5066x; 1.1717x over previous
"""Multi-head self-attention (B=4, T=2048, D=1024, H=16) on 8 TRN2 NeuronCores.

Sharding: core c = 2*b + j computes batch b, heads j*8..j*8+7 (tensor-parallel
over heads), and a partial projection over its 512 attention-output columns.
The host sums the two partial projections per batch. No collectives.

Per-core dataflow (all matmul inputs bf16, fp32 PSUM accumulation):
  - Heads processed in even/odd pairs. K^T and Q^T for a pair live in one
    [128, T] tile (even head's 64 features on partitions 0:64, odd on 64:128).
  - Transposed scores per pair: the even and odd head matmuls contract over
    64 partitions each and run CONCURRENTLY on PE row groups 0-1 / 2-3 via
    tile_position (0,0)/(64,0), writing adjacent PSUM banks [128, 512|512].
  - One exp ACT per kt covers both heads ([128,1024], scale=1/8 folded in).
  - attn@V per head: V_aug [t,65] (ones column -> softmax denominator in
    row 64), accumulated over kt.
  - Normalize via DVE reciprocal_approx_fast + GpSimd partition broadcast +
    DVE multiply into A^T[d, t] (bf16), then partial projection (fp32).
  - The scalar engine (exp) is the bottleneck: QKV-projection and output-
    projection matmul chunks are woven between score slots so exp work is
    available from ~15us onward and the PE never bubbles ACT.
"""

import os
from collections import deque

import numpy as np
import ml_dtypes

import concourse.mybir as mybir
from concourse import bacc
from concourse.tile import TileContext
from concourse.bass_utils import run_bass_kernel_spmd

B, T, D, H = 4, 2048, 1024, 16
HD = D // H
SCALE = HD**-0.5
P = 128
BF = mybir.dt.bfloat16
F32 = mybir.dt.float32
NBF = ml_dtypes.bfloat16

LAST_RESULT = None
_built = None


def _build():
    nc = bacc.Bacc("TRN2", target_bir_lowering=False, debug=False, num_devices=8)

    xT = nc.dram_tensor("xT", [D, T], BF, kind="ExternalInput")  # x[b].T
    wqkT = nc.dram_tensor("wqkT", [D, 1024], BF, kind="ExternalInput")  # (q|k).T shard
    wvT = nc.dram_tensor("wvT", [D, 512], BF, kind="ExternalInput")
    wpT = nc.dram_tensor("wpT", [512, D], BF, kind="ExternalInput")  # proj_w.T rows
    qkb = nc.dram_tensor("qkb", [1024], F32, kind="ExternalInput")
    vb = nc.dram_tensor("vb", [512], F32, kind="ExternalInput")
    pb = nc.dram_tensor("pb", [D], F32, kind="ExternalInput")
    yT = nc.dram_tensor("yT", [D, T], F32, kind="ExternalOutput")

    Exp = mybir.ActivationFunctionType.Exp
    mult = mybir.AluOpType.mult
    add = mybir.AluOpType.add

    with TileContext(nc) as tc:
        with (
            tc.tile_pool(name="pers", bufs=1) as pers,
            tc.tile_pool(name="small", bufs=1) as small,
        ):
            # ---- persistent tensors ----
            # K^T / Q^T feature tiles, one per head pair (even head rows 0:64,
            # odd head rows 64:128)
            kts = [pers.tile([P, T], BF, tag=f"kt{i}", name=f"kt{i}") for i in range(4)]
            qts = [pers.tile([P, T], BF, tag=f"qt{i}", name=f"qt{i}") for i in range(4)]
            # attention out, d-major, one tile per head pair
            ats = [pers.tile([P, T], BF, tag=f"at{i}", name=f"at{i}") for i in range(4)]
            V4 = pers.tile([P, 16, 8, HD + 1], BF, tag="v4")  # [t-part, tt, head, 65]

            # long-lived attention pools first: pools release in LIFO order,
            # and the phase-1 pools below must close mid-emission
            spool_cm = tc.tile_pool(name="spool", bufs=2, space="PSUM")
            spool = spool_cm.__enter__()
            epool_cm = tc.tile_pool(name="epool", bufs=2)
            epool = epool_cm.__enter__()
            pop_cm = tc.tile_pool(name="pop", bufs=1, space="PSUM")
            pop = pop_cm.__enter__()
            npool_cm = tc.tile_pool(name="npool", bufs=2)
            npool = npool_cm.__enter__()

            # ---- input loads ----
            ph1x_cm = tc.tile_pool(name="ph1x", bufs=1)
            ph1x = ph1x_cm.__enter__()
            ph1w_cm = tc.tile_pool(name="ph1w", bufs=1)
            ph1w = ph1w_cm.__enter__()
            xts = []
            wqks = []
            for dt in range(8):
                t_ = ph1x.tile([P, T], BF, tag=f"xt{dt}")
                nc.sync.dma_start(t_[:], xT.ap()[dt * P : (dt + 1) * P, :])
                xts.append(t_)
                w_ = ph1w.tile([P, 1024], BF, tag=f"wqk{dt}")
                nc.sync.dma_start(w_[:], wqkT.ap()[dt * P : (dt + 1) * P, :])
                wqks.append(w_)
            wvs = []
            for dt in range(8):
                w_ = ph1x.tile([P, 512], BF, tag=f"wv{dt}")
                nc.sync.dma_start(w_[:], wvT.ap()[dt * P : (dt + 1) * P, :])
                wvs.append(w_)
            qkb_sb = small.tile([P, 8], F32, tag="qkb")
            nc.sync.dma_start(qkb_sb[:], qkb.rearrange("(o p) -> p o", p=P))
            vb_sb = small.tile([P, 512], F32, tag="vb")
            nc.sync.dma_start(vb_sb[:], vb.ap()[None, :].to_broadcast((P, 512)))
            pb_sb = small.tile([P, 8], F32, tag="pb")
            nc.sync.dma_start(pb_sb[:], pb.rearrange("(o p) -> p o", p=P))
            # ones columns of V_aug
            nc.vector.memset(V4[:, :, :, HD : HD + 1], 1.0)

            ph1p_cm = tc.tile_pool(name="ph1p", bufs=2, space="PSUM")
            ph1p = ph1p_cm.__enter__()

            def qk_chunk(ft, c4):
                # one [128, 512] column chunk of the q/k projection, feature
                # tile ft (0-3 -> Q pair ft, 4-7 -> K pair ft-4)
                p5 = ph1p.tile([P, 512], F32, tag="p5")
                col = c4 * 512
                for dt in range(8):
                    nc.tensor.matmul(
                        p5[:],
                        lhsT=wqks[dt][:, ft * P : (ft + 1) * P],
                        rhs=xts[dt][:, col : col + 512],
                        start=(dt == 0),
                        stop=(dt == 7),
                    )
                dst = qts[ft] if ft < 4 else kts[ft - 4]
                nc.vector.tensor_scalar_add(
                    dst[:, col : col + 512], p5[:], qkb_sb[:, ft : ft + 1]
                )

            def v_chunk(tt):
                p5 = ph1p.tile([P, 512], F32, tag="p5")
                for dt in range(8):
                    nc.tensor.matmul(
                        p5[:],
                        lhsT=xts[dt][:, tt * P : (tt + 1) * P],
                        rhs=wvs[dt][:],
                        start=(dt == 0),
                        stop=(dt == 7),
                    )
                nc.vector.tensor_tensor(
                    V4[:, tt, :, 0:HD],
                    p5.rearrange("p (h e) -> p h e", e=HD),
                    vb_sb.rearrange("p (h e) -> p h e", e=HD),
                    add,
                )

            # K0's first column block + Q0's first q-block upfront so pair 0's
            # scores start as soon as the x DMA lands; everything else is
            # woven between score slots in dependency order (consumer
            # iteration i needs its K/Q/V chunks emitted in iterations < i).
            qk_chunk(4, 0)
            qk_chunk(0, 0)

            def qk_f(ft, c4):
                return lambda: qk_chunk(ft, c4)

            vch = lambda tt: (lambda: v_chunk(tt))  # noqa: E731
            fill0 = [qk_f(4, 1), qk_f(4, 2), qk_f(4, 3)]  # rest of K0 (kt>=4)
            fill0 += [qk_f(5, c) for c in range(4)] + [qk_f(1, 0)]  # K1, Q1c0
            fill0 += [vch(tt) for tt in range(8)]  # V0-7
            # V8-11 first (attn@V(p0) kt8-11 land at slots 12-15 of it1)
            fill1 = [vch(8), vch(9), vch(10), vch(11)]
            fill1 += [qk_f(6, c) for c in range(4)] + [qk_f(2, 0)]  # K2, Q2c0
            fill1 += [vch(12), vch(13), vch(14), vch(15)]
            fill1 += [qk_f(0, 1), qk_f(0, 2)]
            fill2 = [qk_f(7, c) for c in range(4)] + [qk_f(3, 0)]  # K3, Q3c0
            fill2 += [qk_f(0, 3)] + [qk_f(1, c) for c in (1, 2, 3)] + [qk_f(2, 1)]
            fill3 = [qk_f(2, 2), qk_f(2, 3)] + [qk_f(3, c) for c in (1, 2, 3)]
            per_iter_fill = [fill0, fill1, fill2, fill3]

            # ---- attention ----
            wps = [None] * 4
            outp = [None]
            late_cms = []
            proj_q = deque()

            def attnv_slot(st, kt):
                e_, poE_, poO_, hp_, _ = st
                nc.tensor.matmul(
                    poE_[0 : HD + 1, :],
                    lhsT=V4[:, kt, 2 * hp_, :],
                    rhs=e_[:, kt, 0:512],
                    start=(kt == 0),
                    stop=(kt == 15),
                )
                nc.tensor.matmul(
                    poO_[0 : HD + 1, :],
                    lhsT=V4[:, kt, 2 * hp_ + 1, :],
                    rhs=e_[:, kt, 512:1024],
                    start=(kt == 0),
                    stop=(kt == 15),
                )

            def emit_norm(st):
                e_, poE_, poO_, hp_, qc_ = st
                cols = slice(qc_ * 512, (qc_ + 1) * 512)
                for po_, rows in ((poE_, slice(0, 64)), (poO_, slice(64, 128))):
                    # custom-DVE reciprocal_approx_fast can't read PSUM;
                    # stage the denominator row through SBUF first
                    dsb = npool.tile([1, 512], F32, tag="dsb")
                    nc.vector.tensor_copy(dsb[:], po_[HD : HD + 1, :])
                    rr = npool.tile([1, 512], F32, tag="rr")
                    nc.vector.reciprocal_approx_fast(rr[:], dsb[:])
                    rb = npool.tile([64, 512], F32, tag="rb")
                    nc.gpsimd.partition_broadcast(rb[:], rr[:])
                    nc.vector.tensor_tensor(
                        ats[hp_][rows, cols], po_[0:HD, :], rb[:], mult
                    )

            def proj_chunk(qc, ep):
                # et pair (2*ep, 2*ep+1) of the output projection at q-block qc
                pp = spool.tile([P, 1024], F32, tag="ps")
                cols = slice(qc * 512, (qc + 1) * 512)
                for half in range(2):
                    et = 2 * ep + half
                    for dt in range(4):
                        nc.tensor.matmul(
                            pp[:, half * 512 : (half + 1) * 512],
                            lhsT=wps[dt][:, et * P : (et + 1) * P],
                            rhs=ats[dt][:, cols],
                            start=(dt == 0),
                            stop=(dt == 3),
                        )
                ob = outp[0].tile([P, 1024], F32, tag="ob")
                for half in range(2):
                    et = 2 * ep + half
                    hcols = slice(half * 512, (half + 1) * 512)
                    nc.vector.tensor_scalar_add(
                        ob[:, hcols], pp[:, hcols], pb_sb[:, et : et + 1]
                    )
                    nc.sync.dma_start(
                        yT.ap()[et * P : (et + 1) * P, cols], ob[:, hcols]
                    )

            prev = None
            pop2 = [None]
            iters = [(hp, qc) for qc in range(4) for hp in range(4)]
            for it, (hp, qc) in enumerate(iters):
                e = epool.tile([P, 16, 1024], BF, tag="e")
                # alternate between two single-buffered psum pools so the
                # norm chain of iteration i never blocks attn@V of i+1
                pool_i = pop2[0] if (pop2[0] is not None and it % 2 == 1) else pop
                poE = pool_i.tile([P, 512], F32, tag="poe")
                poO = pool_i.tile([P, 512], F32, tag="poo")
                qcols = slice(qc * 512, (qc + 1) * 512)
                fill = deque(per_iter_fill[it]) if it < len(per_iter_fill) else None
                for kt in range(16):
                    # emit ready work (attn@V / qkv / proj) BEFORE the score
                    # matmuls: scores can block on the exp ring (spool WAR),
                    # and the PE queue is in-order
                    if prev is not None and kt >= 4:
                        attnv_slot(prev, kt - 4)
                    if fill:
                        fill.popleft()()
                    if proj_q and hp != 0 and kt in (7, 11):
                        proj_q.popleft()()
                    ps = spool.tile([P, 1024], F32, tag="ps")
                    nc.tensor.matmul(
                        ps[:, 0:512],
                        lhsT=kts[hp][0:64, kt * P : (kt + 1) * P],
                        rhs=qts[hp][0:64, qcols],
                        start=True,
                        stop=True,
                        tile_position=(0, 0),
                    )
                    nc.tensor.matmul(
                        ps[:, 512:1024],
                        lhsT=kts[hp][64:128, kt * P : (kt + 1) * P],
                        rhs=qts[hp][64:128, qcols],
                        start=True,
                        stop=True,
                        tile_position=(64, 0),
                    )
                    nc.scalar.activation(e[:, kt, :], ps[:], Exp, scale=SCALE)
                while fill:
                    fill.popleft()()
                if prev is not None:
                    for kt in range(12, 16):
                        attnv_slot(prev, kt)
                    emit_norm(prev)
                prev = (e, poE, poO, hp, qc)

                if it == 3:
                    # all phase-1 chunks are emitted; free their pools and
                    # bring in the projection weights
                    ph1p_cm.__exit__(None, None, None)
                    ph1w_cm.__exit__(None, None, None)
                    ph1x_cm.__exit__(None, None, None)
                    wpool_cm = tc.tile_pool(name="wpool", bufs=1)
                    wpool = wpool_cm.__enter__()
                    late_cms.append(wpool_cm)
                    for dt in range(4):
                        w_ = wpool.tile([P, D], BF, tag=f"wp{dt}")
                        nc.sync.dma_start(w_[:], wpT.ap()[dt * P : (dt + 1) * P, :])
                        wps[dt] = w_
                    outp_cm = tc.tile_pool(name="outp", bufs=2)
                    outp[0] = outp_cm.__enter__()
                    late_cms.append(outp_cm)
                    pop2_cm = tc.tile_pool(name="pop2", bufs=1, space="PSUM")
                    pop2[0] = pop2_cm.__enter__()
                    late_cms.append(pop2_cm)

                if hp == 3:
                    # queue the projection for this q-block (runs woven into
                    # the next q-block's iterations, after emit_norm(qc, p3))
                    for ep in range(4):
                        proj_q.append(lambda qc=qc, ep=ep: proj_chunk(qc, ep))

            # drain: last iteration's attn@V + norm + last projection block
            for kt in range(16):
                attnv_slot(prev, kt)
            emit_norm(prev)
            while proj_q:
                proj_q.popleft()()

            for cm in [*reversed(late_cms), npool_cm, pop_cm, epool_cm, spool_cm]:
                cm.__exit__(None, None, None)

    nc.compile()
    return nc


def kernel(x, qkv_w, qkv_b, proj_w, proj_b):
    global _built, LAST_RESULT
    x = np.asarray(x, np.float32)
    qkv_w = np.asarray(qkv_w, np.float32)
    qkv_b = np.asarray(qkv_b, np.float32)
    proj_w = np.asarray(proj_w, np.float32)
    proj_b = np.asarray(proj_b, np.float32)

    if _built is None:
        _built = _build()
    nc = _built

    in_maps = []
    for c in range(8):
        b, j = divmod(c, 2)
        s = j * 512
        wqkT = np.concatenate([qkv_w[s : s + 512], qkv_w[1024 + s : 1024 + s + 512]]).T
        in_maps.append(
            {
                "xT": np.ascontiguousarray(x[b].T).astype(NBF),
                "wqkT": np.ascontiguousarray(wqkT).astype(NBF),
                "wvT": np.ascontiguousarray(qkv_w[2048 + s : 2048 + s + 512].T).astype(NBF),
                "wpT": np.ascontiguousarray(proj_w[:, s : s + 512].T).astype(NBF),
                "qkb": np.concatenate([qkv_b[s : s + 512], qkv_b[1024 + s : 1024 + s + 512]]),
                "vb": np.ascontiguousarray(qkv_b[2048 + s : 2048 + s + 512]),
                "pb": proj_b if j == 0 else np.zeros_like(proj_b),
            }
        )

    trace = os.environ.get("BASS_TRACE") == "1"
    if trace:
        try:
            import antenv.axon_hooks  # noqa: F401  (needed by the axon trace path)
        except ImportError:
            trace = False
            os.environ["BASS_NEVER_TRACE"] = "1"
    res = run_bass_kernel_spmd(nc, in_maps, core_ids=list(range(8)), trace=trace)
    LAST_RESULT = res

    out = np.empty((B, T, D), np.float32)
    for b in range(B):
        out[b] = (res.results[2 * b]["yT"] + res.results[2 * b + 1]["yT"]).T
    return out
